# revision 25
# baseline (speedup 1.0000x reference)
"""Trainium2 Bass kernel for nn_Attention_86165633892896 (sparse_attention).

Math: the reference scatters fresh k/v rows into a paged KV cache at
collision-free slots, then immediately gathers the same slots back out.
With unique slots, gather(scatter(cache, s, x), s) == x exactly, so the
cache round-trip is an identity and the output depends only on q, k, v:

    out[b] = softmax(Q_b @ K_b^T * scale) @ V_b        (per batch b)

with Q_b, K_b, V_b of shape [32, 128]  (32 heads, head_dim 128), B = 4096.

Scores are bounded (|s| < ~6 for randn inputs), so softmax without
max-subtraction is numerically safe and matches jax.nn.softmax.

Mapping to one NeuronCore (data-parallel over B, 512 batches/core):
  * batches are processed in "groups" of 4 -> a [128, 128] tile whose
    partition axis is (b_local*32 + head) and free axis is head_dim d.
  * Q,K chunks are loaded FULLY CONTIGUOUSLY (partition p holds gpc
    consecutive rows -> multi-KB DMA descriptors); the PE transposes that
    put d on partitions also repair the layout (see transpose comment).
  * all matmul DATAPATHS run in bf16 (fp32 matmul costs 4 cycles/row on
    the TRN2 PE vs 1 for bf16; rel-err tolerance is 2e-2, bf16 costs
    ~4e-3).  Casts are cheap: qt/kt are downcast by the PSUM->SBUF
    copies that follow the fp32 PE transposes, P is downcast by the
    exp's output, and V is cast by a gpsimd tensor_copy on the
    otherwise-idle Pool engine.
  * QK^T: 4 column-tiled matmuls (tile_position=(0,32j)), one per batch,
    stationary = Q^T[:, 32j:32j+32], moving = K^T[:, 32j:32j+32], bf16.
    Output lands compactly as PSUM [128=(4b,h), 32=k] fp32.
  * softmax: one ACT exp (scale folded in, bf16 out), one DVE reduce_sum,
    one DVE reciprocal.  1/denominator is folded into the output copy.
  * P^T: one DVE StreamTranspose (in-place 32x32 block transposes, bf16).
  * PV: 4 diagonal-tiled matmuls (tile_position=(32j,32j)), stationary =
    P_j^T [32k, 32h] bf16, moving = natural V rows [32k, 128d] bf16.
  * output: one DVE tensor_tensor multiply by broadcast reciprocal,
    PSUM -> SBUF fp32, then contiguous DMA out.

The problem is HBM-bound (32MB/core at ~350GB/s aggregate ~= 93us; all
compute fits under that), so the structure is tuned to keep the DMA
queues busy and the compute fully overlapped:
  * rings: q+v loads on the SP HWDGE ring, k on the ACT ring, the output
    store on the gpsimd (Pool SWDGE) queue -- a store's wait (last DVE
    write of its chunk) parks the issuing queue, so it lives on the
    queue with no downstream input loads ("defer_out" additionally
    issues chunk c's store at the top of chunk c+1, when the wait is
    almost satisfied).
  * sw_pipe: QK(s+1) is issued before PV(s) so PE isn't parked on the
    ACT->DVE softmax chain (~2us of cross-engine latency); tt_delay
    similarly defers supergroup s's normalize-copy until s+1 so DVE
    never waits on PE's PV.
  * ch_pipe (optional): chunk c+1's DMAs/transposes/copies are emitted
    before chunk c's supergroup pipeline.
Four groups form a "supergroup" sharing single softmax/copy instructions;
chunks of gpc groups share one input-DMA round per tensor.
"""

import numpy as np

B = 4096
H = 32
D = 128
SCALE = 0.08838834764831845
NCORES = 8
NB = B // NCORES  # 512 batches per core

SUP = 4  # groups per supergroup (16 batches)


def build_kernel(nb=NB, gpc=8, loop_T=1, ablate=(), bf16=True, v_cast="pool",
                 q_ring="sp", k_ring="act", v_ring="sp", out_ring="pool",
                 sw_pipe=True, defer_out=True, ch_pipe=True, copies="split",
                 tt_delay=True, in_bufs=5, qk_dma_split=1, pv_dtype="bf16"):
    """Build the per-core Bass kernel for nb batches, gpc groups per DMA chunk.

    loop_T > 1 wraps the whole body in a For_i that repeats it (identical
    work each iteration) -- used only for device-time measurement.
    bf16: run QK/PV matmul datapaths in bfloat16 (casts folded into copies).
    v_cast: "act" / "pool" / "pooldma" -- engine for the V fp32->bf16 cast
        ("pool" = gpsimd tensor_copy; "pooldma" = casting SWDGE DMA, +26us).
    *_ring: "sp" / "act" / "pool" -- which DGE queue triggers each transfer.
        out on "pool" keeps its (late-satisfied) wait off the input rings.
    sw_pipe: issue QK(s+1) before PV(s) so PE never parks on the softmax
        chain and the ACT/DVE pipeline stays fed.
    defer_out: issue chunk c's out-DMA at the top of chunk c+1 so its wait
        (last DVE write of chunk c) is nearly satisfied when the queue
        reaches it (no head-of-line blocking of later transfers).
    """
    import contextlib

    import concourse.bacc as bacc
    import concourse.mybir as mybir
    import concourse.tile as tile
    from concourse.masks import make_identity

    f32 = mybir.dt.float32
    bf = mybir.dt.bfloat16
    f32r = mybir.dt.float32r
    mdt = bf if bf16 else f32  # matmul operand dtype
    # pv_dtype="f32r": PV runs in the PE's reduced-precision fp32 mode, so V
    # feeds the matmul straight from its DMA (no cast stage, no v_f32 tile,
    # and the Pool queue carries only out-store triggers)
    pv_f32r = pv_dtype == "f32r" and bf16
    vdt = f32 if (pv_f32r or not bf16) else mdt
    pdt = f32 if pv_f32r else mdt  # p_t / pt dtype
    ngroups = nb // 4
    assert ngroups % gpc == 0
    nchunk = ngroups // gpc
    assert gpc % SUP == 0
    spc = gpc // SUP  # supergroups per chunk
    rows = nb * H

    nc = bacc.Bacc()
    q_d = nc.declare_dram_parameter("q", [rows, D], f32, isOutput=False)
    k_d = nc.declare_dram_parameter("k", [rows, D], f32, isOutput=False)
    v_d = nc.declare_dram_parameter("v", [rows, D], f32, isOutput=False)
    o_d = nc.declare_dram_parameter("out", [rows, D], f32, isOutput=True)

    # chunk views: fully-contiguous load for q/k: partition p holds gpc
    # consecutive rows (8KB descriptors); v/out keep the strided
    # row-per-partition layout (PV needs V rows k-ordered on partitions)
    assert 32 % gpc == 0 or gpc % 32 == 0
    qv = q_d.rearrange("(c p w) d -> c p (w d)", p=128, w=gpc)
    kv = k_d.rearrange("(c p w) d -> c p (w d)", p=128, w=gpc)
    vv = v_d.rearrange("(c g p) d -> c p g d", p=128, g=gpc)
    ov = o_d.rearrange("(c g p) d -> c p g d", p=128, g=gpc)

    with tile.TileContext(nc) as tc:
        with (
            tc.tile_pool(name="const", bufs=1) as cpool,
            tc.tile_pool(name="inch", bufs=in_bufs) as inpool,
            tc.tile_pool(name="chunk", bufs=3) as chpool,
            tc.tile_pool(name="work", bufs=4) as wpool,
            tc.tile_pool(name="psum", bufs=2, space="PSUM") as pspool,
        ):
            ident = cpool.tile([128, 128], f32)
            make_identity(nc, ident[:])
            # zero-output ldweights absorbs the gpsimd identity-ready wait so
            # no real matmul ever carries it (matmul's S3_LW lowering has a
            # single wait slot); the loaded weights are never used
            nc.tensor.ldweights(ident[:, 0:64].bitcast(mybir.dt.bfloat16))

            if loop_T > 1:
                loop_cm = tc.For_i(
                    0,
                    loop_T,
                    1,
                    hint_engines=(
                        mybir.EngineType.PE,
                        mybir.EngineType.Activation,
                        mybir.EngineType.DVE,
                        mybir.EngineType.SP,
                    ),
                )
            else:
                loop_cm = contextlib.nullcontext()
            ring = {
                "sp": nc.sync,
                "act": nc.scalar,
                "pool": nc.gpsimd,
            }
            with loop_cm:
              o_prev = [None]  # (o_ch tile, chunk index) pending out-DMA

              def flush_out():
                  if o_prev[0] is not None:
                      ring[out_ring].dma_start(ov[o_prev[0][1]], o_prev[0][0][:])
                      o_prev[0] = None

              def chunk_head(c):
                  """DMA triggers, V cast, deferred out flush, PE transposes
                  and the PSUM->SBUF copies that produce qt/kt for chunk c."""
                  q_ch = inpool.tile([128, gpc, D], f32, tag="q_ch")
                  k_ch = inpool.tile([128, gpc, D], f32, tag="k_ch")
                  v_ch = inpool.tile([128, gpc, D], vdt, tag="v_ch")
                  if qk_dma_split > 1:
                      # split q/k loads so completion sems fire per piece and
                      # the first transposes can start mid-window
                      h = gpc // qk_dma_split
                      for i in range(qk_dma_split):
                          ring[q_ring].dma_start(
                              q_ch[:, i * h : (i + 1) * h, :],
                              qv[c, :, i * h * D : (i + 1) * h * D],
                          )
                          ring[k_ring].dma_start(
                              k_ch[:, i * h : (i + 1) * h, :],
                              kv[c, :, i * h * D : (i + 1) * h * D],
                          )
                  else:
                      ring[q_ring].dma_start(q_ch[:], qv[c])
                      ring[k_ring].dma_start(k_ch[:], kv[c])
                  if vdt == f32:
                      ring[v_ring].dma_start(v_ch[:], vv[c])
                  elif v_cast == "pooldma":
                      # gpsimd SWDGE casts fp32->bf16 in flight (slow: +26us)
                      nc.gpsimd.dma_start(v_ch[:], vv[c])
                  else:
                      v_f32 = inpool.tile([128, gpc, D], f32, tag="v_f32")
                      ring[v_ring].dma_start(v_f32[:], vv[c])
                      if v_cast == "pool":
                          nc.gpsimd.tensor_copy(v_ch[:], v_f32[:])
                      else:
                          nc.scalar.copy(v_ch[:], v_f32[:])
                  if defer_out:
                      flush_out()
                  # zero-output ldweights absorb each chunk-DMA wait on PE so
                  # no real matmul carries a DMA wait alongside a slot-release
                  # wait (matmul lowering has one wait slot)
                  nc.tensor.ldweights(q_ch[0:32, 0, 0:64].bitcast(mybir.dt.bfloat16))
                  nc.tensor.ldweights(k_ch[0:32, 0, 0:64].bitcast(mybir.dt.bfloat16))
                  if vdt == f32:
                      nc.tensor.ldweights(v_ch[0:32, 0, 0:64].bitcast(mybir.dt.bfloat16))
                  else:
                      nc.tensor.ldweights(v_ch[0:32, 0, 0:64])

                  if "compute" in ablate:
                      ring[out_ring].dma_start(ov[c], q_ch[:])
                      return None

                  # gpc w-transposes per tensor put d on partitions for the
                  # whole chunk; the PSUM->SBUF copy scatters transpose
                  # column (w, p) to flat column gpc*p + w = global row, so
                  # qt_sb[d, R] is Q^T in natural row order and matmul
                  # operand slices are contiguous single-free-dim APs.
                  # The copies also downcast to bf16 for free.  They run on
                  # the Pool engine by default so a late chunk c+1 cannot
                  # head-of-line-block supergroup work queued on ACT/DVE.
                  qt_sb = chpool.tile([128, 128, gpc], mdt, tag="qt_sb")
                  kt_sb = chpool.tile([128, 128, gpc], mdt, tag="kt_sb")
                  qt_w = qt_sb[:].rearrange("a p w -> a w p")
                  kt_w = kt_sb[:].rearrange("a p w -> a w p")
                  for q4 in range(gpc // SUP):
                      ps_qt = pspool.tile([128, SUP, 128], f32, tag="ps_qt")
                      ps_kt = pspool.tile([128, SUP, 128], f32, tag="ps_kt")
                      for wi in range(SUP):
                          w = q4 * SUP + wi
                          nc.tensor.transpose(
                              ps_qt[:, wi, :], q_ch[:, w, :], ident[:]
                          )
                          nc.tensor.transpose(
                              ps_kt[:, wi, :], k_ch[:, w, :], ident[:]
                          )
                      qsl = qt_w[:, q4 * SUP : (q4 + 1) * SUP, :]
                      ksl = kt_w[:, q4 * SUP : (q4 + 1) * SUP, :]
                      if copies == "pool":
                          nc.gpsimd.tensor_copy(qsl, ps_qt[:])
                          nc.gpsimd.tensor_copy(ksl, ps_kt[:])
                      else:
                          nc.scalar.copy(qsl, ps_qt[:])
                          nc.vector.tensor_copy(ksl, ps_kt[:])
                  qt_f = qt_sb[:].rearrange("a p w -> a (p w)")
                  kt_f = kt_sb[:].rearrange("a p w -> a (p w)")
                  return (c, qt_f, kt_f, v_ch)

              def chunk_sgs(st):
                  """Supergroup pipeline for a prepared chunk."""
                  c, qt_f, kt_f, v_ch = st
                  if defer_out:
                      flush_out()
                  o_ch = chpool.tile([128, gpc, D], f32, tag="o_ch")
                  # tiny first-accessor write: carries o_ch's slot-release
                  # wait (out-DMA of chunk c-2) so the real DVE writes only
                  # wait on PE
                  nc.vector.tensor_copy(o_ch[0:1, 0, 0:1], ident[0:1, 0:1])

                  def qk(s):
                      """16 QK matmuls for supergroup s -> fresh ps_s tile."""
                      ps_s = pspool.tile([128, SUP, 32], f32, tag="ps_s")
                      for gi in range(SUP):
                          for j in range(4):
                              bch = (s * SUP + gi) * 4 + j  # batch in chunk
                              nc.tensor.matmul(
                                  ps_s[32 * j : 32 * j + 32, gi, :],
                                  qt_f[:, 32 * bch : 32 * bch + 32],
                                  kt_f[:, 32 * bch : 32 * bch + 32],
                                  tile_position=(0, 32 * j),
                              )
                      return ps_s

                  # PV operand view: bitcast to f32r for the PE's fast
                  # reduced-precision fp32 mode (identity for bf16)
                  mm = (lambda ap: ap.bitcast(f32r)) if pv_f32r else (lambda ap: ap)

                  tt_prev = [None]  # deferred (ps_o, rec, g0) normalize+copy

                  def flush_tt():
                      if tt_prev[0] is not None:
                          ps_o_p, rec_p, g0_p = tt_prev[0]
                          nc.vector.tensor_tensor(
                              o_ch[:, g0_p : g0_p + SUP, :],
                              ps_o_p[:],
                              rec_p[:, :, None].to_broadcast([128, SUP, D]),
                              mybir.AluOpType.mult,
                          )
                          tt_prev[0] = None

                  ps_s_next = qk(0)
                  for s in range(spc):
                      g0 = s * SUP
                      ps_s = ps_s_next

                      p_t = wpool.tile([128, SUP, 32], pdt, tag="p_t")
                      # first-accessor absorber: carries p_t's slot-release
                      # wait (DVE StreamTranspose of supergroup s-2)
                      nc.scalar.copy(p_t[0:1, 0, 0:1], ident[0:1, 0:1])
                      nc.scalar.activation(
                          p_t[:],
                          ps_s[:],
                          mybir.ActivationFunctionType.Exp,
                          scale=SCALE,
                      )
                      den = wpool.tile([128, SUP], f32, tag="den")
                      nc.vector.reduce_sum(
                          den[:], p_t[:], axis=mybir.AxisListType.X
                      )
                      rec = wpool.tile([128, SUP], f32, tag="rec")
                      nc.vector.reciprocal(rec[:], den[:])

                      # software pipeline: issue QK(s+1) before PV(s) so PE
                      # isn't parked on the softmax chain of supergroup s
                      if sw_pipe and s + 1 < spc:
                          ps_s_next = qk(s + 1)

                      pt = wpool.tile([128, SUP, 32], pdt, tag="pt")
                      # first-accessor absorber: carries pt's slot-release
                      # wait (PE PV matmuls of supergroup s-2)
                      nc.vector.tensor_copy(pt[0:1, 0, 0:1], ident[0:1, 0:1])
                      nc.vector.transpose(
                          pt[:].rearrange("p g k -> p (g k)"),
                          p_t[:].rearrange("p g k -> p (g k)"),
                      )

                      # deferred ttmult of supergroup s-1: its PV finished a
                      # supergroup ago, so DVE doesn't park waiting on PE
                      if tt_delay:
                          flush_tt()

                      ps_o = pspool.tile([128, SUP, D], f32, tag="ps_o")
                      if "pv" in ablate:
                          for gi in range(SUP):
                              nc.tensor.matmul(
                                  ps_o[0:32, gi, :],
                                  mm(pt[0:32, gi, :]),
                                  mm(v_ch[0:32, g0 + gi, :]),
                                  tile_position=(0, 0),
                              )
                      else:
                          for gi in range(SUP):
                              for j in range(4):
                                  nc.tensor.matmul(
                                      ps_o[32 * j : 32 * j + 32, gi, :],
                                      mm(pt[32 * j : 32 * j + 32, gi, :]),
                                      mm(v_ch[32 * j : 32 * j + 32, g0 + gi, :]),
                                      tile_position=(32 * j, 32 * j),
                                  )

                      if not sw_pipe and s + 1 < spc:
                          ps_s_next = qk(s + 1)

                      tt_prev[0] = (ps_o, rec, g0)
                      if not tt_delay:
                          flush_tt()
                  flush_tt()

                  if defer_out:
                      o_prev[0] = (o_ch, c)
                  else:
                      ring[out_ring].dma_start(ov[c], o_ch[:])

              prev = None
              for c in range(nchunk):
                  st = chunk_head(c)
                  if st is None:
                      continue
                  if ch_pipe:
                      if prev is not None:
                          chunk_sgs(prev)
                      prev = st
                  else:
                      chunk_sgs(st)
              if prev is not None:
                  chunk_sgs(prev)
              flush_out()

    nc.finalize()
    return nc


_NC_CACHE = {}


def _get_nc(nb=NB, gpc=8):
    key = (nb, gpc)
    if key not in _NC_CACHE:
        _NC_CACHE[key] = build_kernel(nb, gpc)
    return _NC_CACHE[key]


_FN_CACHE = {}


def _get_callable():
    """Compiled 8-core executable + device-resident zero output buffers,
    cached across kernel() calls (a fresh jit/shard_map per call costs ~1-2s
    of host-side retrace)."""
    if "fn" in _FN_CACHE:
        return _FN_CACHE["fn"]
    import jax
    from jax.sharding import Mesh, PartitionSpec
    from jax.experimental.shard_map import shard_map
    from concourse import bass2jax, mybir
    from concourse.bass2jax import _bass_exec_p, partition_id_tensor

    nc = _get_nc()
    bass2jax.install_neuronx_cc_hook()
    partition_name = nc.partition_id_tensor.name if nc.partition_id_tensor else None
    in_names, out_names, out_avals, zero_outs = [], [], [], []
    for alloc in nc.m.functions[0].allocations:
        if not isinstance(alloc, mybir.MemoryLocationSet):
            continue
        name = alloc.memorylocations[0].name
        if alloc.kind == "ExternalInput":
            if name != partition_name:
                in_names.append(name)
        elif alloc.kind == "ExternalOutput":
            out_names.append(name)
            shape = tuple(alloc.tensor_shape)
            dtype = mybir.dt.np(alloc.dtype)
            out_avals.append(jax.core.ShapedArray(shape, dtype))
            zero_outs.append(np.zeros(shape, dtype))
    assert in_names == ["q", "k", "v"], in_names
    all_in_names = list(in_names) + list(out_names)
    if partition_name is not None:
        all_in_names.append(partition_name)

    def _body(*args):
        operands = list(args)
        if partition_name is not None:
            operands.append(partition_id_tensor())
        return tuple(
            _bass_exec_p.bind(
                *operands,
                out_avals=tuple(out_avals),
                in_names=tuple(all_in_names),
                out_names=tuple(out_names),
                lowering_input_output_aliases=(),
                sim_require_finite=True,
                sim_require_nnan=True,
                nc=nc,
            )
        )

    devices = jax.devices()[:NCORES]
    mesh = Mesh(np.asarray(devices), ("core",))
    n_in = len(in_names) + len(zero_outs)
    fn = jax.jit(
        shard_map(
            _body,
            mesh=mesh,
            in_specs=(PartitionSpec("core"),) * n_in,
            out_specs=(PartitionSpec("core"),) * len(out_names),
            check_rep=False,
        ),
        keep_unused=True,
    )
    sh = jax.sharding.NamedSharding(mesh, PartitionSpec("core"))
    dev_zero = [
        jax.device_put(np.concatenate([z] * NCORES, axis=0), sh) for z in zero_outs
    ]
    _FN_CACHE["fn"] = (fn, sh, dev_zero)
    return _FN_CACHE["fn"]


def kernel(q, k, v, k_cache, v_cache, slot_mapping):
    """Full-input entry point: shards batch across 8 cores, returns full output."""
    import jax

    fn, sh, dev_zero = _get_callable()
    glb = lambda a: jax.device_put(
        np.ascontiguousarray(np.asarray(a, dtype=np.float32)).reshape(
            NCORES * NB * H, D
        ),
        sh,
    )
    out = fn(glb(q), glb(k), glb(v), *dev_zero)
    return np.asarray(out[0]).reshape(B, H * D)


# revision 28
# speedup vs baseline: 1.0779x; 1.0779x over previous
"""Trainium2 Bass kernel for nn_Attention_86165633892896 (sparse_attention).

Math: the reference scatters fresh k/v rows into a paged KV cache at
collision-free slots, then immediately gathers the same slots back out.
With unique slots, gather(scatter(cache, s, x), s) == x exactly, so the
cache round-trip is an identity and the output depends only on q, k, v:

    out[b] = softmax(Q_b @ K_b^T * scale) @ V_b        (per batch b)

with Q_b, K_b, V_b of shape [32, 128]  (32 heads, head_dim 128), B = 4096.

Scores are bounded (|s| < ~6 for randn inputs), so softmax without
max-subtraction is numerically safe and matches jax.nn.softmax.

Mapping to one NeuronCore (data-parallel over B, 512 batches/core):
  * batches are processed in "groups" of 4 -> a [128, 128] tile whose
    partition axis is (b_local*32 + head) and free axis is head_dim d.
  * Q,K chunks are loaded FULLY CONTIGUOUSLY (partition p holds gpc
    consecutive rows -> multi-KB DMA descriptors); the PE transposes that
    put d on partitions also repair the layout (see transpose comment).
  * all matmul DATAPATHS run in bf16 (fp32 matmul costs 4 cycles/row on
    the TRN2 PE vs 1 for bf16; rel-err tolerance is 2e-2, bf16 costs
    ~4e-3).  Casts are cheap: qt/kt are downcast by the PSUM->SBUF
    copies that follow the fp32 PE transposes, P is downcast by the
    exp's output, and V is cast by a gpsimd tensor_copy on the
    otherwise-idle Pool engine.
  * QK^T: 4 column-tiled matmuls (tile_position=(0,32j)), one per batch,
    stationary = Q^T[:, 32j:32j+32], moving = K^T[:, 32j:32j+32], bf16.
    Output lands compactly as PSUM [128=(4b,h), 32=k] fp32.
  * softmax: one ACT exp (scale folded in, bf16 out), one DVE reduce_sum,
    one DVE reciprocal.  1/denominator is folded into the output copy.
  * P^T: one DVE StreamTranspose (in-place 32x32 block transposes, bf16).
  * PV: 4 diagonal-tiled matmuls (tile_position=(32j,32j)), stationary =
    P_j^T [32k, 32h] bf16, moving = natural V rows [32k, 128d] bf16.
  * output: one DVE tensor_tensor multiply by broadcast reciprocal,
    PSUM -> SBUF fp32, then contiguous DMA out.

The problem is HBM-bound (32MB/core at ~350GB/s aggregate ~= 93us; all
compute fits under that), so the structure is tuned to keep the DMA
queues busy and the compute fully overlapped:
  * rings: q+v loads on the SP HWDGE ring, k on the ACT ring, the output
    store on the gpsimd (Pool SWDGE) queue -- a store's wait (last DVE
    write of its chunk) parks the issuing queue, so it lives on the
    queue with no downstream input loads ("defer_out" additionally
    issues chunk c's store at the top of chunk c+1, when the wait is
    almost satisfied).
  * sw_pipe: QK(s+1) is issued before PV(s) so PE isn't parked on the
    ACT->DVE softmax chain (~2us of cross-engine latency); tt_delay
    similarly defers supergroup s's normalize-copy until s+1 so DVE
    never waits on PE's PV.
  * ch_pipe (optional): chunk c+1's DMAs/transposes/copies are emitted
    before chunk c's supergroup pipeline.
Four groups form a "supergroup" sharing single softmax/copy instructions;
chunks of gpc groups share one input-DMA round per tensor.
"""

import numpy as np

B = 4096
H = 32
D = 128
SCALE = 0.08838834764831845
NCORES = 8
NB = B // NCORES  # 512 batches per core

SUP = 4  # groups per supergroup (16 batches)


def build_kernel(nb=NB, gpc=8, loop_T=1, ablate=(), bf16=True, v_cast="pool",
                 q_ring="sp", k_ring="act", v_ring="sp", out_ring="pool",
                 sw_pipe=True, defer_out=True, ch_pipe=True, copies="split",
                 tt_delay=True, in_bufs=5, qk_dma_split=2, pv_dtype="bf16",
                 v_split=True):
    """Build the per-core Bass kernel for nb batches, gpc groups per DMA chunk.

    loop_T > 1 wraps the whole body in a For_i that repeats it (identical
    work each iteration) -- used only for device-time measurement.
    bf16: run QK/PV matmul datapaths in bfloat16 (casts folded into copies).
    v_cast: "act" / "pool" / "pooldma" -- engine for the V fp32->bf16 cast
        ("pool" = gpsimd tensor_copy; "pooldma" = casting SWDGE DMA, +26us).
    *_ring: "sp" / "act" / "pool" -- which DGE queue triggers each transfer.
        out on "pool" keeps its (late-satisfied) wait off the input rings.
    sw_pipe: issue QK(s+1) before PV(s) so PE never parks on the softmax
        chain and the ACT/DVE pipeline stays fed.
    defer_out: issue chunk c's out-DMA at the top of chunk c+1 so its wait
        (last DVE write of chunk c) is nearly satisfied when the queue
        reaches it (no head-of-line blocking of later transfers).
    """
    import contextlib

    import concourse.bacc as bacc
    import concourse.mybir as mybir
    import concourse.tile as tile
    from concourse.masks import make_identity

    f32 = mybir.dt.float32
    bf = mybir.dt.bfloat16
    f32r = mybir.dt.float32r
    mdt = bf if bf16 else f32  # matmul operand dtype
    # pv_dtype="f32r": PV runs in the PE's reduced-precision fp32 mode, so V
    # feeds the matmul straight from its DMA (no cast stage, no v_f32 tile,
    # and the Pool queue carries only out-store triggers)
    pv_f32r = pv_dtype == "f32r" and bf16
    vdt = f32 if (pv_f32r or not bf16) else mdt
    pdt = f32 if pv_f32r else mdt  # p_t / pt dtype
    ngroups = nb // 4
    assert ngroups % gpc == 0
    nchunk = ngroups // gpc
    assert gpc % SUP == 0
    spc = gpc // SUP  # supergroups per chunk
    rows = nb * H

    nc = bacc.Bacc()
    q_d = nc.declare_dram_parameter("q", [rows, D], f32, isOutput=False)
    k_d = nc.declare_dram_parameter("k", [rows, D], f32, isOutput=False)
    v_d = nc.declare_dram_parameter("v", [rows, D], f32, isOutput=False)
    o_d = nc.declare_dram_parameter("out", [rows, D], f32, isOutput=True)

    # chunk views: fully-contiguous load for q/k: partition p holds gpc
    # consecutive rows (8KB descriptors); v/out keep the strided
    # row-per-partition layout (PV needs V rows k-ordered on partitions)
    assert 32 % gpc == 0 or gpc % 32 == 0
    qv = q_d.rearrange("(c p w) d -> c p (w d)", p=128, w=gpc)
    kv = k_d.rearrange("(c p w) d -> c p (w d)", p=128, w=gpc)
    vv = v_d.rearrange("(c g p) d -> c p g d", p=128, g=gpc)
    ov = o_d.rearrange("(c g p) d -> c p g d", p=128, g=gpc)

    with tile.TileContext(nc) as tc:
        with (
            tc.tile_pool(name="const", bufs=1) as cpool,
            tc.tile_pool(name="inch", bufs=in_bufs) as inpool,
            tc.tile_pool(name="chunk", bufs=3) as chpool,
            tc.tile_pool(name="work", bufs=4) as wpool,
            tc.tile_pool(name="psum", bufs=2, space="PSUM") as pspool,
        ):
            ident = cpool.tile([128, 128], f32)
            make_identity(nc, ident[:])
            # zero-output ldweights absorbs the gpsimd identity-ready wait so
            # no real matmul ever carries it (matmul's S3_LW lowering has a
            # single wait slot); the loaded weights are never used
            nc.tensor.ldweights(ident[:, 0:64].bitcast(mybir.dt.bfloat16))

            if loop_T > 1:
                loop_cm = tc.For_i(
                    0,
                    loop_T,
                    1,
                    hint_engines=(
                        mybir.EngineType.PE,
                        mybir.EngineType.Activation,
                        mybir.EngineType.DVE,
                        mybir.EngineType.SP,
                    ),
                )
            else:
                loop_cm = contextlib.nullcontext()
            ring = {
                "sp": nc.sync,
                "act": nc.scalar,
                "pool": nc.gpsimd,
            }
            with loop_cm:
              o_prev = [None]  # (o_ch tile, chunk index) pending out-DMA

              def flush_out():
                  if o_prev[0] is not None:
                      ring[out_ring].dma_start(ov[o_prev[0][1]], o_prev[0][0][:])
                      o_prev[0] = None

              def chunk_head(c):
                  """DMA triggers, V cast, deferred out flush, PE transposes
                  and the PSUM->SBUF copies that produce qt/kt for chunk c."""
                  q_ch = inpool.tile([128, gpc, D], f32, tag="q_ch")
                  k_ch = inpool.tile([128, gpc, D], f32, tag="k_ch")
                  v_ch = inpool.tile([128, gpc, D], vdt, tag="v_ch")
                  if qk_dma_split > 1:
                      # split q/k loads so completion sems fire per piece and
                      # the first transposes can start mid-window
                      h = gpc // qk_dma_split
                      for i in range(qk_dma_split):
                          ring[q_ring].dma_start(
                              q_ch[:, i * h : (i + 1) * h, :],
                              qv[c, :, i * h * D : (i + 1) * h * D],
                          )
                          ring[k_ring].dma_start(
                              k_ch[:, i * h : (i + 1) * h, :],
                              kv[c, :, i * h * D : (i + 1) * h * D],
                          )
                  else:
                      ring[q_ring].dma_start(q_ch[:], qv[c])
                      ring[k_ring].dma_start(k_ch[:], kv[c])
                  if vdt == f32:
                      ring[v_ring].dma_start(v_ch[:], vv[c])
                  elif v_cast == "pooldma":
                      # gpsimd SWDGE casts fp32->bf16 in flight (slow: +26us)
                      nc.gpsimd.dma_start(v_ch[:], vv[c])
                  else:
                      v_f32 = inpool.tile([128, gpc, D], f32, tag="v_f32")
                      if v_split:
                          # halve V across two queues: SP runs q+v/2 (12MB)
                          # instead of q+v (16MB) so all three DMA queues
                          # drain together (one queue alone can't saturate
                          # HBM: 115us single vs 95us split, measured)
                          hg = gpc // 2
                          ring[v_ring].dma_start(
                              v_f32[:, 0:hg, :], vv[c][:, 0:hg, :]
                          )
                          nc.gpsimd.dma_start(
                              v_f32[:, hg:, :], vv[c][:, hg:, :]
                          )
                      else:
                          ring[v_ring].dma_start(v_f32[:], vv[c])
                      if v_cast == "pool":
                          nc.gpsimd.tensor_copy(v_ch[:], v_f32[:])
                      else:
                          nc.scalar.copy(v_ch[:], v_f32[:])
                  if defer_out:
                      flush_out()
                  # zero-output ldweights absorb each chunk-DMA wait on PE so
                  # no real matmul carries a DMA wait alongside a slot-release
                  # wait (matmul lowering has one wait slot)
                  nc.tensor.ldweights(q_ch[0:32, 0, 0:64].bitcast(mybir.dt.bfloat16))
                  nc.tensor.ldweights(k_ch[0:32, 0, 0:64].bitcast(mybir.dt.bfloat16))
                  if vdt == f32:
                      nc.tensor.ldweights(v_ch[0:32, 0, 0:64].bitcast(mybir.dt.bfloat16))
                  else:
                      nc.tensor.ldweights(v_ch[0:32, 0, 0:64])

                  if "compute" in ablate:
                      ring[out_ring].dma_start(ov[c], q_ch[:])
                      return None

                  # gpc w-transposes per tensor put d on partitions for the
                  # whole chunk; the PSUM->SBUF copy scatters transpose
                  # column (w, p) to flat column gpc*p + w = global row, so
                  # qt_sb[d, R] is Q^T in natural row order and matmul
                  # operand slices are contiguous single-free-dim APs.
                  # The copies also downcast to bf16 for free.  They run on
                  # the Pool engine by default so a late chunk c+1 cannot
                  # head-of-line-block supergroup work queued on ACT/DVE.
                  qt_sb = chpool.tile([128, 128, gpc], mdt, tag="qt_sb")
                  kt_sb = chpool.tile([128, 128, gpc], mdt, tag="kt_sb")
                  qt_w = qt_sb[:].rearrange("a p w -> a w p")
                  kt_w = kt_sb[:].rearrange("a p w -> a w p")
                  for q4 in range(gpc // SUP):
                      ps_qt = pspool.tile([128, SUP, 128], f32, tag="ps_qt")
                      ps_kt = pspool.tile([128, SUP, 128], f32, tag="ps_kt")
                      for wi in range(SUP):
                          w = q4 * SUP + wi
                          nc.tensor.transpose(
                              ps_qt[:, wi, :], q_ch[:, w, :], ident[:]
                          )
                          nc.tensor.transpose(
                              ps_kt[:, wi, :], k_ch[:, w, :], ident[:]
                          )
                      qsl = qt_w[:, q4 * SUP : (q4 + 1) * SUP, :]
                      ksl = kt_w[:, q4 * SUP : (q4 + 1) * SUP, :]
                      if copies == "pool":
                          nc.gpsimd.tensor_copy(qsl, ps_qt[:])
                          nc.gpsimd.tensor_copy(ksl, ps_kt[:])
                      else:
                          nc.scalar.copy(qsl, ps_qt[:])
                          nc.vector.tensor_copy(ksl, ps_kt[:])
                  qt_f = qt_sb[:].rearrange("a p w -> a (p w)")
                  kt_f = kt_sb[:].rearrange("a p w -> a (p w)")
                  return (c, qt_f, kt_f, v_ch)

              def chunk_sgs(st):
                  """Supergroup pipeline for a prepared chunk."""
                  c, qt_f, kt_f, v_ch = st
                  if defer_out:
                      flush_out()
                  o_ch = chpool.tile([128, gpc, D], f32, tag="o_ch")
                  # tiny first-accessor write: carries o_ch's slot-release
                  # wait (out-DMA of chunk c-2) so the real DVE writes only
                  # wait on PE
                  nc.vector.tensor_copy(o_ch[0:1, 0, 0:1], ident[0:1, 0:1])

                  def qk(s):
                      """16 QK matmuls for supergroup s -> fresh ps_s tile."""
                      ps_s = pspool.tile([128, SUP, 32], f32, tag="ps_s")
                      for gi in range(SUP):
                          for j in range(4):
                              bch = (s * SUP + gi) * 4 + j  # batch in chunk
                              nc.tensor.matmul(
                                  ps_s[32 * j : 32 * j + 32, gi, :],
                                  qt_f[:, 32 * bch : 32 * bch + 32],
                                  kt_f[:, 32 * bch : 32 * bch + 32],
                                  tile_position=(0, 32 * j),
                              )
                      return ps_s

                  # PV operand view: bitcast to f32r for the PE's fast
                  # reduced-precision fp32 mode (identity for bf16)
                  mm = (lambda ap: ap.bitcast(f32r)) if pv_f32r else (lambda ap: ap)

                  tt_prev = [None]  # deferred (ps_o, rec, g0) normalize+copy

                  def flush_tt():
                      if tt_prev[0] is not None:
                          ps_o_p, rec_p, g0_p = tt_prev[0]
                          nc.vector.tensor_tensor(
                              o_ch[:, g0_p : g0_p + SUP, :],
                              ps_o_p[:],
                              rec_p[:, :, None].to_broadcast([128, SUP, D]),
                              mybir.AluOpType.mult,
                          )
                          tt_prev[0] = None

                  ps_s_next = qk(0)
                  for s in range(spc):
                      g0 = s * SUP
                      ps_s = ps_s_next

                      p_t = wpool.tile([128, SUP, 32], pdt, tag="p_t")
                      # first-accessor absorber: carries p_t's slot-release
                      # wait (DVE StreamTranspose of supergroup s-2)
                      nc.scalar.copy(p_t[0:1, 0, 0:1], ident[0:1, 0:1])
                      nc.scalar.activation(
                          p_t[:],
                          ps_s[:],
                          mybir.ActivationFunctionType.Exp,
                          scale=SCALE,
                      )
                      den = wpool.tile([128, SUP], f32, tag="den")
                      nc.vector.reduce_sum(
                          den[:], p_t[:], axis=mybir.AxisListType.X
                      )
                      rec = wpool.tile([128, SUP], f32, tag="rec")
                      nc.vector.reciprocal(rec[:], den[:])

                      # software pipeline: issue QK(s+1) before PV(s) so PE
                      # isn't parked on the softmax chain of supergroup s
                      if sw_pipe and s + 1 < spc:
                          ps_s_next = qk(s + 1)

                      pt = wpool.tile([128, SUP, 32], pdt, tag="pt")
                      # first-accessor absorber: carries pt's slot-release
                      # wait (PE PV matmuls of supergroup s-2)
                      nc.vector.tensor_copy(pt[0:1, 0, 0:1], ident[0:1, 0:1])
                      nc.vector.transpose(
                          pt[:].rearrange("p g k -> p (g k)"),
                          p_t[:].rearrange("p g k -> p (g k)"),
                      )

                      # deferred ttmult of supergroup s-1: its PV finished a
                      # supergroup ago, so DVE doesn't park waiting on PE
                      if tt_delay:
                          flush_tt()

                      ps_o = pspool.tile([128, SUP, D], f32, tag="ps_o")
                      if "pv" in ablate:
                          for gi in range(SUP):
                              nc.tensor.matmul(
                                  ps_o[0:32, gi, :],
                                  mm(pt[0:32, gi, :]),
                                  mm(v_ch[0:32, g0 + gi, :]),
                                  tile_position=(0, 0),
                              )
                      else:
                          for gi in range(SUP):
                              for j in range(4):
                                  nc.tensor.matmul(
                                      ps_o[32 * j : 32 * j + 32, gi, :],
                                      mm(pt[32 * j : 32 * j + 32, gi, :]),
                                      mm(v_ch[32 * j : 32 * j + 32, g0 + gi, :]),
                                      tile_position=(32 * j, 32 * j),
                                  )

                      if not sw_pipe and s + 1 < spc:
                          ps_s_next = qk(s + 1)

                      tt_prev[0] = (ps_o, rec, g0)
                      if not tt_delay:
                          flush_tt()
                  flush_tt()

                  if defer_out:
                      o_prev[0] = (o_ch, c)
                  else:
                      ring[out_ring].dma_start(ov[c], o_ch[:])

              prev = None
              for c in range(nchunk):
                  st = chunk_head(c)
                  if st is None:
                      continue
                  if ch_pipe:
                      if prev is not None:
                          chunk_sgs(prev)
                      prev = st
                  else:
                      chunk_sgs(st)
              if prev is not None:
                  chunk_sgs(prev)
              flush_out()

    nc.finalize()
    return nc


_NC_CACHE = {}


def _get_nc(nb=NB, gpc=8):
    key = (nb, gpc)
    if key not in _NC_CACHE:
        _NC_CACHE[key] = build_kernel(nb, gpc)
    return _NC_CACHE[key]


_FN_CACHE = {}


def _get_callable():
    """Compiled 8-core executable + device-resident zero output buffers,
    cached across kernel() calls (a fresh jit/shard_map per call costs ~1-2s
    of host-side retrace)."""
    if "fn" in _FN_CACHE:
        return _FN_CACHE["fn"]
    import jax
    from jax.sharding import Mesh, PartitionSpec
    from jax.experimental.shard_map import shard_map
    from concourse import bass2jax, mybir
    from concourse.bass2jax import _bass_exec_p, partition_id_tensor

    nc = _get_nc()
    bass2jax.install_neuronx_cc_hook()
    partition_name = nc.partition_id_tensor.name if nc.partition_id_tensor else None
    in_names, out_names, out_avals, zero_outs = [], [], [], []
    for alloc in nc.m.functions[0].allocations:
        if not isinstance(alloc, mybir.MemoryLocationSet):
            continue
        name = alloc.memorylocations[0].name
        if alloc.kind == "ExternalInput":
            if name != partition_name:
                in_names.append(name)
        elif alloc.kind == "ExternalOutput":
            out_names.append(name)
            shape = tuple(alloc.tensor_shape)
            dtype = mybir.dt.np(alloc.dtype)
            out_avals.append(jax.core.ShapedArray(shape, dtype))
            zero_outs.append(np.zeros(shape, dtype))
    assert in_names == ["q", "k", "v"], in_names
    all_in_names = list(in_names) + list(out_names)
    if partition_name is not None:
        all_in_names.append(partition_name)

    def _body(*args):
        operands = list(args)
        if partition_name is not None:
            operands.append(partition_id_tensor())
        return tuple(
            _bass_exec_p.bind(
                *operands,
                out_avals=tuple(out_avals),
                in_names=tuple(all_in_names),
                out_names=tuple(out_names),
                lowering_input_output_aliases=(),
                sim_require_finite=True,
                sim_require_nnan=True,
                nc=nc,
            )
        )

    devices = jax.devices()[:NCORES]
    mesh = Mesh(np.asarray(devices), ("core",))
    n_in = len(in_names) + len(zero_outs)
    fn = jax.jit(
        shard_map(
            _body,
            mesh=mesh,
            in_specs=(PartitionSpec("core"),) * n_in,
            out_specs=(PartitionSpec("core"),) * len(out_names),
            check_rep=False,
        ),
        keep_unused=True,
    )
    sh = jax.sharding.NamedSharding(mesh, PartitionSpec("core"))
    dev_zero = [
        jax.device_put(np.concatenate([z] * NCORES, axis=0), sh) for z in zero_outs
    ]
    _FN_CACHE["fn"] = (fn, sh, dev_zero)
    return _FN_CACHE["fn"]


def kernel(q, k, v, k_cache, v_cache, slot_mapping):
    """Full-input entry point: shards batch across 8 cores, returns full output."""
    import jax

    fn, sh, dev_zero = _get_callable()
    glb = lambda a: jax.device_put(
        np.ascontiguousarray(np.asarray(a, dtype=np.float32)).reshape(
            NCORES * NB * H, D
        ),
        sh,
    )
    out = fn(glb(q), glb(k), glb(v), *dev_zero)
    return np.asarray(out[0]).reshape(B, H * D)
